# revision 21
# baseline (speedup 1.0000x reference)
"""GATv2ConvNet on 8 Trainium2 NeuronCores (Bass/Tile).

Sharding: nodes degree-sorted and dealt round-robin across 8 cores; each core
owns 3840 node rows (3750 real + 90 pads) in 30 tiles of 128. Per tile, the
incoming-edge slots are padded to the tile's max in-degree (common schedule
across cores, ~5% overhead). Edge aggregation per destination tile:
  t = gather(xl, src) + xr[dst]            (indirect DMA w/ CCE-add accumulate)
  logit = att . lrelu(t); p = exp(logit + mask)
  out = (sum_j p*t)/(sum_j p) - xr + bo    (so raw xl[src] is never needed)
Layer 0 gathers directly from emb@W tables with CPU-composed indices.
Between layers, locally computed xl rows are AllGathered. Pooling uses
is_equal masks + PSUM-accumulated matmuls, then AllReduce + the MLP head.
"""
import numpy as np

P = 128
NCORES = 8
N = 30000
NLP = 3840            # padded nodes per core
TPC = 30              # tiles per core
NP = NCORES * NLP
V = 10000
VT = 79               # emb tiles
VP = VT * P           # padded vocab rows
G = 128
OOB = 1 << 24
LAYERS = [dict(H=3, C=32), dict(H=2, C=96), dict(H=1, C=64)]

_CACHE = {}


def _prep(inp):
    node_ids = np.asarray(inp["node_ids"], np.int64)
    ei = np.asarray(inp["edge_index"], np.int64)
    batch = np.asarray(inp["batch"], np.int64)

    loops = np.arange(N, dtype=np.int64)
    src = np.concatenate([ei[0], loops])
    dst = np.concatenate([ei[1], loops])
    deg = np.bincount(dst, minlength=N)

    order = np.argsort(deg, kind="stable")
    rank_of_old = np.empty(N, np.int64)
    rank_of_old[order] = np.arange(N)
    PADC = NLP - N // NCORES
    new_of_old = (rank_of_old % NCORES) * NLP + PADC + rank_of_old // NCORES
    old_of_new = np.full(NP, -1, np.int64)
    old_of_new[new_of_old] = np.arange(N)

    newdeg = np.zeros(NP, np.int64)
    newdeg[new_of_old] = deg
    Dsched = np.zeros(TPC, np.int64)
    for t in range(TPC):
        m = 1
        for c in range(NCORES):
            b = c * NLP + t * P
            m = max(m, newdeg[b:b + P].max())
        Dsched[t] = m
    offs = np.concatenate([[0], np.cumsum(Dsched)]).astype(np.int64)
    SD = int(offs[-1])

    # edges bucketed by new dst
    src_new = new_of_old[src]
    dst_new = new_of_old[dst]
    eo = np.argsort(dst_new, kind="stable")
    sb, db = src_new[eo], dst_new[eo]
    starts = np.searchsorted(db, np.arange(NP))
    ends = np.searchsorted(db, np.arange(NP) + 1)

    per_core = []
    nid_new = np.zeros(NP, np.int64)
    nid_new[new_of_old] = node_ids

    def wrap16(flat):
        # dma_gather index layout: flat i -> [i % 16, i // 16], tiled to 128
        w = flat.astype(np.int16).reshape(-1, 16).T.copy()
        return np.tile(w, (8, 1))

    for c in range(NCORES):
        nbr = np.zeros((P, SD), np.int64)       # slot j of node p (new ids)
        mask = np.full((P, SD), -30.0, np.float32)
        batf = np.full((P, TPC), 200.0, np.float32)
        for t in range(TPC):
            D = int(Dsched[t])
            o = int(offs[t])
            for p in range(P):
                n = c * NLP + t * P + p
                s0, s1 = int(starts[n]), int(ends[n])
                k = s1 - s0
                nbr[p, o:o + k] = sb[s0:s1]
                mask[p, o:o + k] = 0.0
                if old_of_new[n] >= 0:
                    batf[p, t] = float(batch[old_of_new[n]])
        # j-major flat order per tile, concatenated: [128, 8*SD] int16
        l12_cols, l0_cols = [], []
        for t in range(TPC):
            D = int(Dsched[t])
            o = int(offs[t])
            blk = nbr[:, o:o + D]               # [P, D]
            flat = blk.T.ravel()                # i = j*128 + p
            l12_cols.append(wrap16(flat))
            l0_cols.append(wrap16(nid_new[flat]))
        l12 = np.concatenate(l12_cols, axis=1)
        l0 = np.concatenate(l0_cols, axis=1)
        # xr0: row of local node (t, p) at flat i = t*128 + p
        nloc = np.arange(NLP) + c * NLP
        xr0 = wrap16(nid_new[nloc])
        per_core.append(dict(l12_idx=l12, l0_idx=l0, slot_mask=mask,
                             xr0_idx=xr0, batch_f=batf))

    f32 = lambda k: np.ascontiguousarray(np.asarray(inp[k], np.float32))
    tile128 = lambda v: np.ascontiguousarray(np.tile(v.reshape(1, -1), (P, 1)))

    embT = np.zeros((16, VP), np.float32)
    embT[:, :V] = f32("emb").T
    shared = dict(
        embT=embT,
        Wl0=f32("Wl0"), Wr0=f32("Wr0"),
        Wl1=f32("Wl1"), Wr1=f32("Wr1"),
        Wl2=f32("Wl2"), Wr2=f32("Wr2"),
        bl0_t=np.pad(tile128(f32("bl0")), ((0, 0), (0, 32))),
        br0_t=np.pad(tile128(f32("br0")), ((0, 0), (0, 32))),
        bl1_t=tile128(f32("bl1")), br1_t=tile128(f32("br1")),
        bl2_t=tile128(f32("bl2")), br2_t=tile128(f32("br2")),
        bo0_t=tile128(f32("bo0")), bo1_t=tile128(f32("bo1")),
        bo2_t=tile128(f32("bo2")),
        att0_t=tile128(f32("att0").reshape(-1)),
        att1_t=tile128(f32("att1").reshape(-1)),
        att2_t=tile128(f32("att2").reshape(-1)),
        Wc1=f32("Wc1"), Wc2=f32("Wc2"),
        bc1_t=tile128(f32("bc1")), bc2_t=tile128(f32("bc2")),
        demoT=np.ascontiguousarray(f32("demographics").T),
        inv_cnt=np.ascontiguousarray(
            (1.0 / np.maximum(np.bincount(batch, minlength=G), 1)
             ).astype(np.float32).reshape(G, 1)),
        iota=np.ascontiguousarray(
            np.tile(np.arange(P, dtype=np.float32).reshape(1, P), (P, 1))),
        ident=np.eye(P, dtype=np.float32),
    )
    in_maps = [dict(shared, **pc) for pc in per_core]
    return in_maps, Dsched.tolist(), [int(x) for x in offs], SD


def _build(Dsched, offs, SD, stage=3):
    import concourse.bass as bass
    import concourse.bacc as bacc
    import concourse.tile as tile
    from concourse import mybir
    from concourse.bass import IndirectOffsetOnAxis

    dt = mybir.dt.float32
    AF = mybir.ActivationFunctionType
    OP = mybir.AluOpType
    AX = mybir.AxisListType
    Dmax = max(Dsched)

    nc = bacc.Bacc("TRN2", target_bir_lowering=False, debug=False,
                   num_devices=NCORES)

    def din(name, shape, d=dt):
        return nc.dram_tensor(name, list(shape), d, kind="ExternalInput")

    i_embT = din("embT", [16, VP])
    i_W = {k: din(k, s) for k, s in [
        ("Wl0", (16, 96)), ("Wr0", (16, 96)),
        ("Wl1", (96, 192)), ("Wr1", (96, 192)),
        ("Wl2", (192, 64)), ("Wr2", (192, 64)),
        ("Wc1", (69, 32)), ("Wc2", (32, 2))]}
    i_bt = {k: din(k, (P, n)) for k, n in [
        ("bl0_t", 128), ("br0_t", 128), ("bl1_t", 192), ("br1_t", 192),
        ("bl2_t", 64), ("br2_t", 64), ("bo0_t", 96), ("bo1_t", 192),
        ("bo2_t", 64), ("att0_t", 96), ("att1_t", 192), ("att2_t", 64),
        ("bc1_t", 32), ("bc2_t", 2)]}
    i_demoT = din("demoT", [5, P])
    i_invc = din("inv_cnt", [G, 1])
    i_iota = din("iota", [P, P])
    i_ident = din("ident", [P, P])
    i_l12 = din("l12_idx", [P, 8 * SD], mybir.dt.int16)
    i_l0 = din("l0_idx", [P, 8 * SD], mybir.dt.int16)
    i_mask = din("slot_mask", [P, SD])
    i_xr0 = din("xr0_idx", [P, NLP // 16], mybir.dt.int16)
    i_batf = din("batch_f", [P, TPC])

    o_out = nc.dram_tensor("out", [G, 2], dt, kind="ExternalOutput")
    o_dbg = (nc.dram_tensor("dbg", [NLP, 192], dt, kind="ExternalOutput")
             if stage <= 1 else None)
    o_dbg2 = (nc.dram_tensor("dbg2", [NP, 192], dt, kind="ExternalOutput")
              if stage == 2 else None)

    with tile.TileContext(nc) as tc:
        with tc.tile_pool(name="cpool", bufs=1) as cpool, \
             tc.tile_pool(name="xt", bufs=1) as xtp, \
             tc.tile_pool(name="tp", bufs=2) as tp, \
             tc.tile_pool(name="lrp", bufs=1) as lrp, \
             tc.tile_pool(name="sp", bufs=4) as sp, \
             tc.tile_pool(name="os", bufs=3) as osp, \
             tc.tile_pool(name="ps", bufs=2, space="PSUM") as psp, \
             tc.tile_pool(name="pool_ps", bufs=1, space="PSUM") as poolp, \
             tc.tile_pool(name="dram", bufs=1, space="DRAM") as dram:

            # ---- load constants ----
            def load(ap_src, shape, d=dt, name=None):
                t = cpool.tile(list(shape), d, name=name, tag=name)
                nc.sync.dma_start(t[:], ap_src)
                return t

            W = {}
            for k, v in i_W.items():
                if v.shape[0] > P:
                    W[k + "_a"] = load(v.ap()[0:P, :], (P, v.shape[1]), name=k + "_a")
                    W[k + "_b"] = load(v.ap()[P:, :],
                                       (v.shape[0] - P, v.shape[1]),
                                       name=k + "_b")
                else:
                    W[k] = load(v.ap(), v.shape, name=k)
            BT = {k: load(v.ap(), v.shape, name='c' + k) for k, v in i_bt.items()}
            demoT = load(i_demoT.ap(), (5, P), name='cdemoT')
            invc = load(i_invc.ap(), (G, 1), name='cinvc')
            iota = load(i_iota.ap(), (P, P), name='ciota')
            ident = load(i_ident.ap(), (P, P), name='cident')
            l12 = load(i_l12.ap(), (P, 8 * SD), mybir.dt.int16, name='cl12')
            l0i = load(i_l0.ap(), (P, 8 * SD), mybir.dt.int16, name='cl0')
            maskt = load(i_mask.ap(), (P, SD), name='cmask')
            xr0i = load(i_xr0.ap(), (P, NLP // 16), mybir.dt.int16, name='cxr0')
            batf = load(i_batf.ap(), (P, TPC), name='cbatf')

            # ---- internal DRAM ----
            embWl = dram.tile([VP, 128], dt)
            embWr = dram.tile([VP, 128], dt)
            xl_loc = {1: dram.tile([NLP, 192], dt, name="xl_loc1"),
                      2: dram.tile([NLP, 64], dt, name="xl_loc2")}
            xl_full = {1: dram.tile([NP, 192], dt, name="xl_full1"),
                       2: dram.tile([NP, 64], dt, name="xl_full2")}
            pool_in = dram.tile([G, 64], dt)
            pool_out = dram.tile([G, 64], dt)

            # persistent SBUF
            xt_a = xtp.tile([P, NLP], dt)     # rows 0..127 of x^T
            xt_b = xtp.tile([P, NLP], dt)     # rows 128..191 (layer2 input)
            xr_loc = xtp.tile([P, TPC * 192], dt)

            # ---- emb tables ----
            for v in range(VT):
                embt = sp.tile([16, P], dt, tag="embt")
                nc.sync.dma_start(embt[:], i_embT.ap()[:, v * P:(v + 1) * P])
                lhsT = embt[:]
                for Wk, bk, dst_d in (("Wl0", "bl0_t", embWl),
                                      ("Wr0", "br0_t", embWr)):
                    ps = psp.tile([P, 96], dt, tag="mm")
                    nc.tensor.matmul(ps[:], lhsT, W[Wk][:], start=True,
                                     stop=True)
                    xsb = osp.tile([P, 128], dt, tag="xsb0")
                    nc.vector.tensor_tensor(out=xsb[:, 0:96], in0=ps[:],
                                            in1=BT[bk][:, 0:96], op=OP.add)
                    nc.vector.tensor_copy(out=xsb[:, 96:128],
                                          in_=BT[bk][:, 96:128])
                    nc.sync.dma_start(dst_d[v * P:(v + 1) * P, :], xsb[:])

            # xr0: gather embWr rows for local nodes -> xr_loc [p, t, 128]
            nc.gpsimd.dma_gather(
                out_ap=xr_loc[:, :TPC * 128].rearrange(
                    "p (t f) -> p t f", t=TPC),
                in_ap=embWr[:],
                idxs_ap=xr0i[:],
                num_idxs=NLP,
                num_idxs_reg=NLP,
                elem_size=128,
                single_packet=False,
            )

            if stage == 0:
                for t in range(TPC):
                    nc.sync.dma_start(
                        o_dbg.ap()[t * P:(t + 1) * P, 0:128],
                        xr_loc[:, t * 128:(t + 1) * 128])

            pool_acc = poolp.tile([G, 64], dt)

            def _pool_head():
                pool_sb = sp.tile([G, 64], dt, tag="poolsb")
                nc.scalar.copy(pool_sb[:], pool_acc[:])
                nc.sync.dma_start(pool_in[:], pool_sb[:])
                nc.gpsimd.collective_compute(
                    "AllReduce", OP.add,
                    replica_groups=[list(range(NCORES))],
                    ins=[pool_in.opt()], outs=[pool_out.opt()],
                )
                pool_r = sp.tile([G, 64], dt, tag="poolr")
                nc.sync.dma_start(pool_r[:], pool_out[:])
                nc.vector.tensor_scalar_mul(pool_r[:], pool_r[:], invc[:])
                hT = sp.tile([69, P], dt, tag="hT")
                ppt = psp.tile([P, P], dt, tag="tr")
                nc.tensor.transpose(ppt[0:64, :], pool_r[:], ident[:])
                nc.scalar.copy(hT[0:64, :], ppt[0:64, :])
                nc.scalar.copy(hT[64:69, :], demoT[:])
                h1ps = psp.tile([P, 32], dt, tag="tr")
                nc.tensor.matmul(h1ps[:], hT[:], W["Wc1"][:], start=True,
                                 stop=True)
                h1 = sp.tile([P, 32], dt, tag="h1s")
                nc.vector.tensor_tensor(out=h1[:], in0=h1ps[:],
                                        in1=BT["bc1_t"][:], op=OP.add)
                nc.scalar.activation(h1[:], h1[:], AF.Relu)
                h1t = psp.tile([P, P], dt, tag="tr")
                nc.tensor.transpose(h1t[0:32, :], h1[:], ident[:])
                h1T = sp.tile([32, P], dt, tag="h1T")
                nc.scalar.copy(h1T[:], h1t[0:32, :])
                ops_ = psp.tile([P, 2], dt, tag="tr")
                nc.tensor.matmul(ops_[:], h1T[:], W["Wc2"][:], start=True,
                                 stop=True)
                osb = sp.tile([P, 2], dt, tag="ofin")
                nc.vector.tensor_tensor(out=osb[:], in0=ops_[:],
                                        in1=BT["bc2_t"][:], op=OP.add)
                nc.sync.dma_start(o_out.ap(), osb[:])

            nlayers = 0 if stage == 0 else (1 if stage == 1 else 3)
            for ell, cfg in enumerate(LAYERS[:nlayers]):
                H, C = cfg["H"], cfg["C"]
                F = H * C
                attk = f"att{ell}_t"
                bok = f"bo{ell}_t"

                if ell > 0:
                    Fin = 96 if ell == 1 else 192
                    Wlk, Wrk = f"Wl{ell}", f"Wr{ell}"
                    blk, brk = f"bl{ell}_t", f"br{ell}_t"
                    # dense: xl rows -> DRAM, xr -> xr_loc
                    for t in range(TPC):
                        cs = slice(t * P, (t + 1) * P)
                        for (Wk, bk, to_dram) in ((Wlk, blk, True),
                                                  (Wrk, brk, False)):
                            ps = psp.tile([P, F], dt, tag="mm")
                            if Fin == 96:
                                nc.tensor.matmul(ps[:], xt_a[0:96, cs],
                                                 W[Wk][:], start=True,
                                                 stop=True)
                            else:
                                nc.tensor.matmul(ps[:], xt_a[:, cs],
                                                 W[Wk + "_a"][:], start=True,
                                                 stop=False)
                                nc.tensor.matmul(ps[:], xt_b[0:64, cs],
                                                 W[Wk + "_b"][:],
                                                 start=False, stop=True)
                            if to_dram:
                                xsb = osp.tile([P, F], dt)
                                nc.vector.tensor_tensor(
                                    out=xsb[:], in0=ps[:], in1=BT[bk][:],
                                    op=OP.add)
                                nc.sync.dma_start(xl_loc[ell][cs, :], xsb[:])
                            else:
                                nc.vector.tensor_tensor(
                                    out=xr_loc[:, t * F:(t + 1) * F],
                                    in0=ps[:], in1=BT[bk][:], op=OP.add)
                    nc.gpsimd.collective_compute(
                        "AllGather", OP.bypass,
                        replica_groups=[list(range(NCORES))],
                        ins=[xl_loc[ell].opt()], outs=[xl_full[ell].opt()],
                    )
                    if stage == 2 and ell == 1:
                        for t in range(NP // P):
                            dsb = osp.tile([P, 192], dt, tag="dbg2sb")
                            nc.sync.dma_start(
                                dsb[:], xl_full[1][t * P:(t + 1) * P, :])
                            nc.sync.dma_start(
                                o_dbg2.ap()[t * P:(t + 1) * P, :], dsb[:])
                        break
                    gsrc, gidx, EF = xl_full[ell], l12, F
                else:
                    gsrc, gidx, EF = embWl, l0i, 128

                # ---- edge phase ----
                for t in range(TPC):
                    D = Dsched[t]
                    o = offs[t]
                    xr_t = xr_loc[:, t * EF:t * EF + F]
                    tb = tp.tile([P, Dmax * 192], dt, tag="t")
                    tbf = tb[:, :D * EF].rearrange("p (d f) -> p d f", d=D)
                    tbv = tbf[:, :, 0:F]     # [P, D, F] (strided when EF=128)
                    # gather xl rows
                    nc.gpsimd.dma_gather(
                        out_ap=tbf,
                        in_ap=gsrc[:],
                        idxs_ap=gidx[:, 8 * o:8 * (o + D)],
                        num_idxs=P * D,
                        num_idxs_reg=P * D,
                        elem_size=EF,
                        single_packet=False,
                    )
                    # t += xr (broadcast over slots)
                    nc.vector.tensor_tensor(
                        out=tbv, in0=tbv,
                        in1=xr_t.unsqueeze(1).broadcast_to([P, D, F]),
                        op=OP.add)
                    # lrelu
                    lr = lrp.tile([P, Dmax * 192], dt, tag="lr")
                    lrv3 = lr[:, :D * F].rearrange("p (d f) -> p d f", d=D)
                    nc.scalar.activation(lrv3, tbv, AF.Lrelu, alpha=0.2)
                    # att mul (in place on lr)
                    nc.vector.tensor_tensor(
                        out=lrv3, in0=lrv3,
                        in1=BT[attk][:, 0:F].unsqueeze(1).broadcast_to(
                            [P, D, F]),
                        op=OP.mult)
                    # logit = reduce over C
                    logit = sp.tile([P, Dmax * 3], dt, tag="logit")
                    lgv = logit[:, :D * H]
                    nc.vector.tensor_reduce(
                        out=lgv,
                        in_=lr[:, :D * F].rearrange(
                            "p (d h c) -> p d h c", d=D, h=H),
                        axis=AX.X, op=OP.add)
                    # + mask
                    nc.vector.tensor_tensor(
                        out=lgv.rearrange("p (d h) -> p d h", d=D),
                        in0=lgv.rearrange("p (d h) -> p d h", d=D),
                        in1=maskt[:, o:o + D].unsqueeze(2).broadcast_to(
                            [P, D, H]),
                        op=OP.add)
                    # p = exp
                    pt = sp.tile([P, Dmax * 3], dt, tag="p")
                    pv = pt[:, :D * H]
                    nc.scalar.activation(pv, lgv, AF.Exp)
                    # t *= p (bcast over C), in place
                    tb4 = tb[:, :D * EF].rearrange(
                        "p (d ef) -> p d ef", d=D)[:, :, 0:F].rearrange(
                        "p d (h c) -> p d h c", h=H)
                    nc.vector.tensor_tensor(
                        out=tb4, in0=tb4,
                        in1=pv.rearrange("p (d h) -> p d h", d=D)
                            .unsqueeze(3).broadcast_to([P, D, H, C]),
                        op=OP.mult)
                    # acc = reduce over slots
                    acc = sp.tile([P, 192], dt, tag="acc")
                    av = acc[:, :F]
                    nc.vector.tensor_reduce(
                        out=av,
                        in_=tb[:, :D * EF].rearrange(
                            "p (d ef) -> p d ef", d=D)[:, :, 0:F].rearrange(
                            "p d f -> p f d"),
                        axis=AX.X, op=OP.add)
                    # s, r
                    st = sp.tile([P, 3], dt, tag="s")
                    sv = st[:, :H]
                    nc.vector.tensor_reduce(
                        out=sv, in_=pv.rearrange("p (d h) -> p h d", d=D),
                        axis=AX.X, op=OP.add)
                    rt = sp.tile([P, 3], dt, tag="r")
                    rv = rt[:, :H]
                    nc.vector.reciprocal(rv, sv)
                    # out = acc*r - xr + bo
                    ot = osp.tile([P, 192], dt, tag="o")
                    ov = ot[:, :F]
                    nc.vector.tensor_tensor(
                        out=ov.rearrange("p (h c) -> p h c", h=H),
                        in0=av.rearrange("p (h c) -> p h c", h=H),
                        in1=rv.unsqueeze(2).broadcast_to([P, H, C]),
                        op=OP.mult)
                    nc.vector.tensor_tensor(out=ov, in0=ov, in1=xr_t,
                                            op=OP.subtract)
                    nc.vector.tensor_tensor(out=ov, in0=ov, in1=BT[bok][:],
                                            op=OP.add)
                    cs = slice(t * P, (t + 1) * P)
                    if stage == 1:
                        nc.sync.dma_start(o_dbg.ap()[cs, 0:F], ov)
                    if ell < 2:
                        # transpose into next layer's xT
                        for k in range((F + P - 1) // P):
                            rn = min(P, F - k * P)
                            pst = psp.tile([P, P], dt, tag="tr")
                            nc.tensor.transpose(
                                pst[:rn, :], ov[:, k * P:k * P + rn], ident[:])
                            dst_t = xt_a if k == 0 else xt_b
                            nc.scalar.copy(dst_t[0:rn, cs], pst[:rn, :])
                    else:
                        sm = sp.tile([P, P], dt, tag="sm")
                        nc.vector.tensor_tensor(
                            out=sm[:],
                            in0=batf[:, t:t + 1].broadcast_to([P, P]),
                            in1=iota[:], op=OP.is_equal)
                        nc.tensor.matmul(pool_acc[:], sm[:], ov,
                                         start=(t == 0), stop=(t == TPC - 1))

            # ---- pool + head ----
            if stage >= 3:
                _pool_head()

    nc.compile()
    return nc


def kernel(__stage=3, **inputs):
    import concourse.bass_utils as bass_utils

    in_maps, Dsched, offs, SD = _prep(inputs)
    key = ("nc", tuple(Dsched), __stage)
    if key not in _CACHE:
        _CACHE[key] = _build(Dsched, offs, SD, stage=__stage)
    nc = _CACHE[key]
    res = bass_utils.run_bass_kernel_spmd(
        nc, in_maps, core_ids=list(range(NCORES)))
    if __stage < 3:
        return res.results
    return np.asarray(res.results[0]["out"], np.float32)
